# revision 10
# baseline (speedup 1.0000x reference)
"""HGNNConv Trainium2 kernel (8 NeuronCores).

Computes  Y = relu( D_n^{-1/2} H D_e^{-1} H^T D_n^{-1/2} (X W^T + b) )
for a hypergraph given by incidence pairs (node_idx[i], edge_idx[i]).

Strategy (2 NEFFs, 8 cores each):
  Phase A (per core, edges sharded):  gather raw x rows per incidence via
    dma_gather, segment-sum onto hyperedges with one-hot matmuls
    (p[e] = sum s_n x_n, q[e] = sum s_n built from the same scaled one-hot),
    then per 128-edge block:  ef[e,:] = inv_de[e] * (W @ p[e] + q[e] * b).
  Host: concatenates the 8 e_feat slices into one table (pure layout).
  Phase B (per core, nodes sharded):  gather e_feat rows per incidence,
    segment-sum onto nodes with one-hot matmuls, then
    y[n,:] = relu(s_n * sum).  s_n = rsqrt(d_n) computed on device.

Host work is limited to integer index packing / slicing / permutation;
every floating point operation runs on device.
"""
import os
import sys

for _p in ("/opt/trn_rl_repo", "/root/.axon_site/_ro/trn_rl_repo"):
    if os.path.isdir(_p) and _p not in sys.path:
        sys.path.insert(0, _p)

import numpy as np
from contextlib import ExitStack

import concourse.bacc as bacc
import concourse.mybir as mybir
import concourse.tile as tile
from concourse.bass_utils import run_bass_kernel_spmd

P = 128
NCORE = 8
CHUNK = 32768          # dma_gather int16 index reach (rows per chunk)
D = 128                # feature dim (in == out == 128)
GB_B = 2               # node blocks per phase-B gather instruction

# set HGNN_TIME=1 to measure warm device-execute wall time per phase; times
# in ns are appended to LAST_EXEC_NS
LAST_EXEC_NS = []
ALL_EXEC_NS = []
TIME_RUNS = os.environ.get("HGNN_TIME", "0") == "1"


def _with_retry(fn, attempts=3, wait_s=5.0):
    """Retry transient tunnel failures (worker hang-ups) around a device run."""
    import time as _time
    for attempt in range(attempts):
        try:
            return fn()
        except Exception:
            if attempt == attempts - 1:
                raise
            _time.sleep(wait_s)

_NC_CACHE = {}
_JIT_CACHE = {}


def _timed_spmd(nc, in_maps, key):
    """run_bass_via_pjrt equivalent that stages inputs first and times the
    warm execute (jit cached per nc)."""
    import time as _time
    import jax
    from jax.sharding import Mesh, PartitionSpec, NamedSharding
    from jax.experimental.shard_map import shard_map
    from concourse import bass2jax as b2j
    from concourse import mybir as _mb

    n_cores = len(in_maps)
    partition_name = (nc.partition_id_tensor.name
                     if nc.partition_id_tensor else None)
    in_names, out_names, out_avals, zero_outs = [], [], [], []
    for alloc in nc.m.functions[0].allocations:
        if not isinstance(alloc, _mb.MemoryLocationSet):
            continue
        name = alloc.memorylocations[0].name
        if alloc.kind == "ExternalInput":
            if name != partition_name:
                in_names.append(name)
        elif alloc.kind == "ExternalOutput":
            out_names.append(name)
            shape = tuple(alloc.tensor_shape)
            dtype = _mb.dt.np(alloc.dtype)
            out_avals.append(jax.core.ShapedArray(shape, dtype))
            zero_outs.append(np.zeros(shape, dtype))
    n_params = len(in_names)
    all_in_names = in_names + out_names
    if partition_name is not None:
        all_in_names.append(partition_name)

    def _body(*args):
        operands = list(args)
        if partition_name is not None:
            operands.append(b2j.partition_id_tensor())
        return tuple(b2j._bass_exec_p.bind(
            *operands,
            out_avals=tuple(out_avals),
            in_names=tuple(all_in_names),
            out_names=tuple(out_names),
            lowering_input_output_aliases=(),
            sim_require_finite=True,
            sim_require_nnan=True,
            nc=nc,
        ))

    devices = jax.devices()[:n_cores]
    mesh = Mesh(np.asarray(devices), ("core",))
    if key not in _JIT_CACHE:
        _JIT_CACHE[key] = jax.jit(shard_map(
            _body, mesh=mesh,
            in_specs=(PartitionSpec("core"),) * (n_params + len(out_names)),
            out_specs=(PartitionSpec("core"),) * len(out_names),
            check_rep=False))
    fn = _JIT_CACHE[key]

    sh = NamedSharding(mesh, PartitionSpec("core"))
    staged = [jax.device_put(
        np.concatenate([np.asarray(m[nm]) for m in in_maps], axis=0), sh)
        for nm in in_names]
    staged += [jax.device_put(
        np.zeros((n_cores * z.shape[0], *z.shape[1:]), z.dtype), sh)
        for z in zero_outs]
    jax.block_until_ready(staged)

    out = fn(*staged)            # cold: trace + compile + execute
    jax.block_until_ready(out)
    best_ns = None
    for _ in range(8):           # warm: execute only; min-of-8 denoises
        t0 = _time.perf_counter()
        out = fn(*staged)
        jax.block_until_ready(out)
        dt_ns = int((_time.perf_counter() - t0) * 1e9)
        ALL_EXEC_NS.append(dt_ns)
        best_ns = dt_ns if best_ns is None else min(best_ns, dt_ns)
    LAST_EXEC_NS.append(best_ns)

    class _R:
        pass
    r = _R()
    r.results = [
        {nm: np.asarray(out[i]).reshape(n_cores, *out_avals[i].shape)[c]
         for i, nm in enumerate(out_names)}
        for c in range(n_cores)
    ]
    return r


# ----------------------------------------------------------------- packing --
def _wrap_idx16(idx, pad_to):
    """int array -> [128, pad_to/16] int16 tile in dma_gather wrap order."""
    n = len(idx)
    assert n <= pad_to and pad_to % 16 == 0
    full = np.zeros(pad_to, np.int16)
    full[:n] = idx.astype(np.int16)
    arr16 = full.reshape(-1, 16).T            # [16, pad_to/16]
    return np.tile(arr16, (8, 1))             # replicate for 8 q7 cores


def _pack_edges(node_idx, edge_idx, N, E, d_n, d_e):
    """Assign edges to (core, block, slot) bins; group incidences by node
    chunk inside each block.  Returns per-core phase-A arrays + edge slots."""
    nbins = None
    n_chunks = (N + CHUNK - 1) // CHUNK
    EBLK = -(-E // (NCORE * P))               # edge blocks per core
    nbins = NCORE * EBLK

    chunk_of_inc = (node_idx // CHUNK).astype(np.int64)
    # per-edge per-chunk incidence counts
    prof = np.zeros((E, n_chunks), np.int64)
    np.add.at(prof, (edge_idx, chunk_of_inc), 1)
    tot_per_chunk = prof.sum(axis=0)

    caps = np.maximum(128, ((-(-tot_per_chunk // nbins) + 127) // 128) * 128)

    order = np.argsort(-d_e, kind="stable")
    for _attempt in range(6):
        loads = np.zeros((nbins, n_chunks), np.int64)
        ecnt = np.zeros(nbins, np.int64)
        bin_of_edge = np.full(E, -1, np.int64)
        ok = True
        capsf = caps.astype(np.float64)
        for e in order:
            pe = prof[e]
            cand = loads + pe
            feas = (cand <= caps).all(axis=1) & (ecnt < P)
            if not feas.any():
                ok = False
                break
            score = (cand / capsf).max(axis=1) + ecnt / (P * 4.0)
            score[~feas] = np.inf
            b = int(np.argmin(score))
            bin_of_edge[e] = b
            loads[b] += pe
            ecnt[b] += 1
        if ok:
            break
        caps = caps + 128                      # bump every chunk cap, retry
    assert ok, "edge packing failed"

    slot_in_bin = np.zeros(E, np.int64)
    cnt = np.zeros(nbins, np.int64)
    for e in order:                            # same order as assignment
        b = bin_of_edge[e]
        slot_in_bin[e] = cnt[b]
        cnt[b] += 1

    # global edge slot (row in the ef table)
    eslot = bin_of_edge * P + slot_in_bin      # [E]

    caps = caps.astype(np.int64)
    T_A = int(caps.sum()) // P                 # tiles per block
    cap_off = np.concatenate([[0], np.cumsum(caps)])  # intra-block offsets

    # destination slot for every incidence
    ebin = bin_of_edge[edge_idx]               # [NI]
    # rank incidences within (bin, chunk)
    key = ebin * n_chunks + chunk_of_inc
    sort = np.argsort(key, kind="stable")
    ks = key[sort]
    # position within group
    grp_start = np.searchsorted(ks, np.arange(nbins * n_chunks), side="left")
    grp_sizes = np.diff(np.concatenate([grp_start, [len(ks)]]))
    assert (grp_sizes.reshape(nbins, n_chunks) <= caps[None, :]).all()
    pos_in_grp = np.arange(len(ks)) - grp_start[ks]
    # slot index within the block's T_A*128 incidence region
    blk_pos = cap_off[ks % n_chunks] + pos_in_grp
    binid = ks // n_chunks
    core = binid // EBLK
    blk = binid % EBLK

    SLOTS = EBLK * T_A * P                     # incidence slots per core
    idxA = np.zeros((NCORE, SLOTS), np.int64)  # chunk-local node idx
    segA = np.full((NCORE, SLOTS), -1.0, np.float32)
    cntA = np.ones((NCORE, SLOTS), np.int64)

    flat = blk * (T_A * P) + blk_pos           # slot within core
    inc_sorted = sort                          # incidence ids in group order
    n_loc = node_idx[inc_sorted] - (ks % n_chunks) * CHUNK
    idxA[core, flat] = n_loc
    segA[core, flat] = slot_in_bin[edge_idx[inc_sorted]].astype(np.float32)
    cntA[core, flat] = d_n[node_idx[inc_sorted]]

    # cntE: [core][P, EBLK] edge sizes per slot (1 for empty slots)
    cntE = np.ones((NCORE, P, EBLK), np.int64)
    cntE[eslot // (EBLK * P), eslot % P, (eslot // P) % EBLK] = np.maximum(d_e, 1)

    # idx wrap layout: each (block, chunk) region (contiguous, multiple of
    # 128 slots) is independently wrapped into [16, w/16] then replicated.
    idxA_w = np.zeros((NCORE, P, SLOTS // 16), np.int16)
    for c in range(NCORE):
        out = np.zeros((P, SLOTS // 16), np.int16)
        col0 = 0
        for b in range(EBLK):
            base = b * T_A * P
            for ch in range(n_chunks):
                w = int(caps[ch])              # slots in this region
                vals = idxA[c, base + cap_off[ch]:base + cap_off[ch] + w]
                arr16 = vals.reshape(-1, 16).T.astype(np.int16)
                out[:, col0:col0 + w // 16] = np.tile(arr16, (8, 1))
                col0 += w // 16
        idxA_w[c] = out

    # seg/cnt in [128, T] tile-major layout
    segA_t = segA.reshape(NCORE, EBLK * T_A, P).transpose(0, 2, 1).copy()
    cntA_t = cntA.reshape(NCORE, EBLK * T_A, P).transpose(0, 2, 1).copy()

    return dict(EBLK=EBLK, T_A=T_A, caps=caps, cap_off=cap_off,
                n_chunks=n_chunks, idxA=idxA_w, segA=segA_t, cntA=cntA_t,
                cntE=cntE, eslot=eslot)


def _pack_nodes(node_idx, edge_idx, N, d_n, eslot):
    """Assign nodes to (core, block, slot); incidences grouped by node."""
    # distribute nodes over cores by degree (snake over sorted degrees)
    order = np.argsort(-d_n, kind="stable")
    core_of_node = np.zeros(N, np.int64)
    core_load = np.zeros(NCORE, np.int64)
    core_ncnt = np.zeros(NCORE, np.int64)
    # snake assignment
    idx = np.arange(N)
    snake = np.empty(N, np.int64)
    rounds = -(-N // NCORE)
    pos = 0
    for r in range(rounds):
        blkn = order[r * NCORE:(r + 1) * NCORE]
        if r % 2:
            blkn = blkn[::-1]
        snake[pos:pos + len(blkn)] = blkn
        pos += len(blkn)
    core_seq = np.tile(np.concatenate([np.arange(NCORE), np.arange(NCORE)[::-1]]),
                       rounds // 2 + 1)[:N]
    core_of_node[snake] = core_seq

    TB = 8
    for _ in range(4):
        ok = True
        NBLK = max(1, -(-max(np.bincount(core_of_node, minlength=NCORE).max(), 1)
                        // P))
        # find per-core packing with cap TB*128 incidences, 128 nodes
        while True:
            blk_of_node = np.full(N, -1, np.int64)
            slot_of_node = np.full(N, -1, np.int64)
            ok = True
            for c in range(NCORE):
                nodes = np.where(core_of_node == c)[0]
                deg = d_n[nodes]
                o = np.argsort(-deg, kind="stable")
                nodes = nodes[o]
                deg = deg[o]
                loads = np.zeros(NBLK, np.int64)
                ncnt = np.zeros(NBLK, np.int64)
                bless = np.arange(NBLK)
                for n, dg in zip(nodes, deg):
                    feas = (loads + dg <= TB * P) & (ncnt < P)
                    if not feas.any():
                        ok = False
                        break
                    b = bless[feas][np.argmin(loads[feas])]
                    blk_of_node[n] = b
                    slot_of_node[n] = ncnt[b]
                    loads[b] += dg
                    ncnt[b] += 1
                if not ok:
                    break
            if ok:
                break
            NBLK += 1
            if NBLK > 2 * (-(-N // (NCORE * P))) + 8:
                break
        if ok:
            break
        TB += 1
    assert ok, "node packing failed"

    # incidence destinations
    n_of_inc = node_idx
    c_of_inc = core_of_node[n_of_inc]
    b_of_inc = blk_of_node[n_of_inc]
    key = c_of_inc * NBLK + b_of_inc
    sort = np.argsort(key, kind="stable")
    ks = key[sort]
    grp_start = np.searchsorted(ks, np.arange(NCORE * NBLK), side="left")
    pos_in_grp = np.arange(len(ks)) - grp_start[ks]

    SLOTS = NBLK * TB * P
    idxB = np.zeros((NCORE, SLOTS), np.int64)
    segB = np.full((NCORE, SLOTS), -1.0, np.float32)

    core = ks // NBLK
    blk = ks % NBLK
    flat = blk * (TB * P) + pos_in_grp
    assert (pos_in_grp < TB * P).all()
    idxB[core, flat] = eslot[edge_idx[sort]]
    segB[core, flat] = slot_of_node[node_idx[sort]].astype(np.float32)

    cntB = np.ones((NCORE, P, NBLK), np.int64)
    valid = blk_of_node >= 0
    cntB[core_of_node[valid], slot_of_node[valid], blk_of_node[valid]] = \
        np.maximum(d_n[valid], 1)

    idxB_w = np.zeros((NCORE, P, SLOTS // 16), np.int16)
    for c in range(NCORE):
        arr16 = idxB[c].reshape(-1, 16).T.astype(np.int16)
        idxB_w[c] = np.tile(arr16, (8, 1))
    segB_t = segB.reshape(NCORE, NBLK * TB, P).transpose(0, 2, 1).copy()

    return dict(NBLK=NBLK, TB=TB, idxB=idxB_w, segB=segB_t, cntB=cntB,
                core_of_node=core_of_node, blk_of_node=blk_of_node,
                slot_of_node=slot_of_node)


# ----------------------------------------------------------------- kernels --
def _build_neff_a(N, EBLK, T_A, caps, cap_off, n_chunks):
    nc = bacc.Bacc("TRN2", target_bir_lowering=False, debug=False,
                   num_devices=NCORE)
    f32, i16, i32 = mybir.dt.float32, mybir.dt.int16, mybir.dt.int32
    f16 = mybir.dt.float16
    TA_tot = EBLK * T_A
    SLOTS = TA_tot * P

    x = nc.dram_tensor("x", [N, D], f16, kind="ExternalInput")
    wt = nc.dram_tensor("wt", [D, D], f16, kind="ExternalInput")     # W.T
    # constf: [0:128]=iota, [128]=ones, [129]=bias row broadcast... bias is a
    # row [1,128]; store at col 129..256 on partition 0 only is awkward ->
    # separate [1,128] tensor.
    constf = nc.dram_tensor("constf", [P, P + 1], f32, kind="ExternalInput")
    bias = nc.dram_tensor("bias", [1, D], f16, kind="ExternalInput")
    idxA = nc.dram_tensor("idxA", [P, SLOTS // 16], i16, kind="ExternalInput")
    segA = nc.dram_tensor("segA", [P, TA_tot], f32, kind="ExternalInput")
    cntA = nc.dram_tensor("cntA", [P, TA_tot], i32, kind="ExternalInput")
    cntE = nc.dram_tensor("cntE", [P, EBLK], i32, kind="ExternalInput")
    ef = nc.dram_tensor("ef", [EBLK * P, D], f32, kind="ExternalOutput")

    with tile.TileContext(nc) as tc, ExitStack() as ctx:
        const = ctx.enter_context(tc.tile_pool(name="const", bufs=1))
        gpool = ctx.enter_context(tc.tile_pool(name="gather", bufs=3))
        ohpool = ctx.enter_context(tc.tile_pool(name="onehot", bufs=12))
        epool = ctx.enter_context(tc.tile_pool(name="efp", bufs=2))
        opool = ctx.enter_context(tc.tile_pool(name="out", bufs=2))
        pp = ctx.enter_context(tc.tile_pool(name="pp", bufs=3, space="PSUM"))
        pq = ctx.enter_context(tc.tile_pool(name="pq", bufs=2, space="PSUM"))
        pe = ctx.enter_context(tc.tile_pool(name="pe", bufs=2, space="PSUM"))

        constf_t = const.tile([P, P + 1], f32)
        nc.sync.dma_start(constf_t[:], constf[:])
        iota = constf_t[:, 0:P]
        ones = constf_t[:, P:P + 1]
        wt_t = const.tile([P, D], f16)
        nc.sync.dma_start(wt_t[:], wt[:])
        b_t = const.tile([1, D], f16)
        nc.sync.dma_start(b_t[:], bias[:])
        ones16 = const.tile([P, 1], f16)
        nc.vector.memset(ones16[:], 1.0)
        idx_t = const.tile([P, SLOTS // 16], i16)
        nc.sync.dma_start(idx_t[:], idxA[:])
        seg_t = const.tile([P, TA_tot], f32)
        nc.sync.dma_start(seg_t[:], segA[:])
        cnta_t = const.tile([P, TA_tot], i32)
        nc.sync.dma_start(cnta_t[:], cntA[:])
        cnte_t = const.tile([P, EBLK], i32)
        nc.sync.dma_start(cnte_t[:], cntE[:])

        # s per incidence = sqrt(1/cnt); inv_de = 1/cntE
        s_t = const.tile([P, TA_tot], f32)
        nc.vector.tensor_copy(s_t[:], cnta_t[:])           # i32 -> f32
        nc.vector.reciprocal(s_t[:], s_t[:])
        nc.scalar.sqrt(s_t[:], s_t[:])
        inv_de = const.tile([P, EBLK], f32)
        nc.vector.tensor_copy(inv_de[:], cnte_t[:])
        nc.vector.reciprocal(inv_de[:], inv_de[:])

        co = [int(v) // P for v in cap_off]    # region offsets in tiles
        for blk in range(EBLK):
            g = gpool.tile([P, T_A, D], f16, tag="g")
            base16 = blk * (T_A * P) // 16
            for ch in range(n_chunks):
                cw = int(caps[ch])             # slots in this region
                lo = CHUNK * ch
                hi = min(N, CHUNK * (ch + 1))
                nc.gpsimd.dma_gather(
                    out_ap=g[:, co[ch]:co[ch + 1], :],
                    in_ap=x[lo:hi, :],
                    idxs_ap=idx_t[:, base16 + int(cap_off[ch]) // 16:
                                  base16 + int(cap_off[ch + 1]) // 16],
                    num_idxs=cw,
                    num_idxs_reg=cw,
                    elem_size=D,
                    single_packet=(cw <= 1024),
                )
            psum_p = pp.tile([P, P], f32, tag="pp")
            psum_q = pq.tile([1, P], f32, tag="pq")
            for t in range(T_A):
                tt = blk * T_A + t
                oh = ohpool.tile([P, P], f16, tag="oh")
                nc.vector.tensor_scalar(
                    out=oh[:], in0=iota,
                    scalar1=seg_t[:, tt:tt + 1], scalar2=s_t[:, tt:tt + 1],
                    op0=mybir.AluOpType.is_equal, op1=mybir.AluOpType.mult)
                nc.tensor.matmul(psum_p[:], lhsT=g[:, t, :], rhs=oh[:],
                                 start=(t == 0), stop=(t == T_A - 1))
                nc.tensor.matmul(psum_q[:], lhsT=ones16[:], rhs=oh[:],
                                 start=(t == 0), stop=(t == T_A - 1))
            efp = epool.tile([P, P], f16, tag="efp")
            nc.scalar.activation(efp[:], psum_p[:],
                                 mybir.ActivationFunctionType.Copy)
            qrow = epool.tile([1, P], f16, tag="qrow")
            nc.vector.tensor_copy(qrow[:], psum_q[:])
            psum_e = pe.tile([P, P], f32, tag="pe")
            nc.tensor.matmul(psum_e[:], lhsT=efp[:], rhs=wt_t[:],
                             start=True, stop=False)
            nc.tensor.matmul(psum_e[:], lhsT=qrow[:], rhs=b_t[:],
                             start=False, stop=True)
            out_t = opool.tile([P, P], f32, tag="out")
            nc.scalar.activation(out_t[:], psum_e[:],
                                 mybir.ActivationFunctionType.Copy,
                                 scale=inv_de[:, blk:blk + 1])
            nc.sync.dma_start(ef[blk * P:(blk + 1) * P, :], out_t[:])
    nc.compile()
    return nc


def _build_neff_b(NSLOT, NBLK, TB):
    nc = bacc.Bacc("TRN2", target_bir_lowering=False, debug=False,
                   num_devices=NCORE)
    f32, i16, i32 = mybir.dt.float32, mybir.dt.int16, mybir.dt.int32
    TB_tot = NBLK * TB
    SLOTS = TB_tot * P

    f16 = mybir.dt.float16
    ef = nc.dram_tensor("ef", [NSLOT, D], f16, kind="ExternalInput")
    constf = nc.dram_tensor("constf", [P, P], f32, kind="ExternalInput")
    idxB = nc.dram_tensor("idxB", [P, SLOTS // 16], i16, kind="ExternalInput")
    segB = nc.dram_tensor("segB", [P, TB_tot], f32, kind="ExternalInput")
    cntB = nc.dram_tensor("cntB", [P, NBLK], i32, kind="ExternalInput")
    y = nc.dram_tensor("y", [NBLK * P, D], f32, kind="ExternalOutput")

    with tile.TileContext(nc) as tc, ExitStack() as ctx:
        const = ctx.enter_context(tc.tile_pool(name="const", bufs=1))
        gpool = ctx.enter_context(tc.tile_pool(name="gather", bufs=3))
        ohpool = ctx.enter_context(tc.tile_pool(name="onehot", bufs=12))
        opool = ctx.enter_context(tc.tile_pool(name="out", bufs=3))
        py = ctx.enter_context(tc.tile_pool(name="py", bufs=4, space="PSUM"))

        constf_t = const.tile([P, P], f32)
        nc.sync.dma_start(constf_t[:], constf[:])
        iota = constf_t[:, 0:P]
        idx_t = const.tile([P, SLOTS // 16], i16)
        nc.sync.dma_start(idx_t[:], idxB[:])
        seg_t = const.tile([P, TB_tot], f32)
        nc.sync.dma_start(seg_t[:], segB[:])
        cntb_t = const.tile([P, NBLK], i32)
        nc.sync.dma_start(cntb_t[:], cntB[:])

        s_t = const.tile([P, NBLK], f32)
        nc.vector.tensor_copy(s_t[:], cntb_t[:])
        nc.vector.reciprocal(s_t[:], s_t[:])
        nc.scalar.sqrt(s_t[:], s_t[:])

        nblk_grp = -(-NBLK // GB_B)
        for grp in range(nblk_grp):
            b0 = grp * GB_B
            nb = min(GB_B, NBLK - b0)
            g = gpool.tile([P, GB_B * TB, D], f16, tag="g")
            nc.gpsimd.dma_gather(
                out_ap=g[:, 0:nb * TB, :],
                in_ap=ef[:],
                idxs_ap=idx_t[:, b0 * TB * P // 16:(b0 + nb) * TB * P // 16],
                num_idxs=nb * TB * P,
                num_idxs_reg=nb * TB * P,
                elem_size=D,
                single_packet=(nb * TB * P <= 1024),
            )
            for bi in range(nb):
                blk = b0 + bi
                psum_y = py.tile([P, P], f32, tag="py")
                for t in range(TB):
                    tt = blk * TB + t
                    oh = ohpool.tile([P, P], f16, tag="oh")
                    nc.vector.tensor_scalar(
                        out=oh[:], in0=iota,
                        scalar1=seg_t[:, tt:tt + 1], scalar2=None,
                        op0=mybir.AluOpType.is_equal)
                    nc.tensor.matmul(psum_y[:], lhsT=oh[:],
                                     rhs=g[:, bi * TB + t, :],
                                     start=(t == 0), stop=(t == TB - 1))
                out_t = opool.tile([P, P], f32, tag="out")
                nc.scalar.activation(out_t[:], psum_y[:],
                                     mybir.ActivationFunctionType.Relu,
                                     scale=s_t[:, blk:blk + 1])
                nc.sync.dma_start(y[blk * P:(blk + 1) * P, :], out_t[:])
    nc.compile()
    return nc


def _build_neff_fused(N, EBLK, T_A, caps, cap_off, n_chunks, NSLOT, NBLK, TB):
    """Single NEFF: phase A (edge shard) -> AllGather(ef) -> phase B (node
    shard).  One dispatch instead of two, so only one host<->device round
    trip is paid."""
    nc = bacc.Bacc("TRN2", target_bir_lowering=False, debug=False,
                   num_devices=NCORE)
    f32, i16, i32 = mybir.dt.float32, mybir.dt.int16, mybir.dt.int32
    f16 = mybir.dt.float16
    TA_tot = EBLK * T_A
    SLOTS_A = TA_tot * P
    TB_tot = NBLK * TB
    SLOTS_B = TB_tot * P

    x = nc.dram_tensor("x", [N, D], f16, kind="ExternalInput")
    wt = nc.dram_tensor("wt", [D, D], f16, kind="ExternalInput")
    constf = nc.dram_tensor("constf", [P, P + 1], f32, kind="ExternalInput")
    bias = nc.dram_tensor("bias", [1, D], f16, kind="ExternalInput")
    idxA = nc.dram_tensor("idxA", [P, SLOTS_A // 16], i16,
                          kind="ExternalInput")
    segA = nc.dram_tensor("segA", [P, TA_tot], f32, kind="ExternalInput")
    cntA = nc.dram_tensor("cntA", [P, TA_tot], i32, kind="ExternalInput")
    cntE = nc.dram_tensor("cntE", [P, EBLK], i32, kind="ExternalInput")
    idxB = nc.dram_tensor("idxB", [P, SLOTS_B // 16], i16,
                          kind="ExternalInput")
    segB = nc.dram_tensor("segB", [P, TB_tot], f32, kind="ExternalInput")
    cntB = nc.dram_tensor("cntB", [P, NBLK], i32, kind="ExternalInput")
    y = nc.dram_tensor("y", [NBLK * P, D], f32, kind="ExternalOutput")

    with tile.TileContext(nc) as tc, ExitStack() as ctx:
        dram = ctx.enter_context(tc.tile_pool(name="dram", bufs=1,
                                              space="DRAM"))
        ef_loc = dram.tile([EBLK * P, D], f16)
        ef_full = dram.tile([NSLOT, D], f16)

        const = ctx.enter_context(tc.tile_pool(name="const", bufs=1))
        gpool = ctx.enter_context(tc.tile_pool(name="gather", bufs=3))
        ohpool = ctx.enter_context(tc.tile_pool(name="onehot", bufs=12))
        epool = ctx.enter_context(tc.tile_pool(name="efp", bufs=2))
        opool = ctx.enter_context(tc.tile_pool(name="out", bufs=2))
        pp = ctx.enter_context(tc.tile_pool(name="pp", bufs=2, space="PSUM"))
        pq = ctx.enter_context(tc.tile_pool(name="pq", bufs=1, space="PSUM"))
        pe = ctx.enter_context(tc.tile_pool(name="pe", bufs=1, space="PSUM"))
        gpoolB = ctx.enter_context(tc.tile_pool(name="gatherB", bufs=3))
        ohpoolB = ctx.enter_context(tc.tile_pool(name="onehotB", bufs=12))
        opoolB = ctx.enter_context(tc.tile_pool(name="outB", bufs=3))
        py = ctx.enter_context(tc.tile_pool(name="py", bufs=4, space="PSUM"))

        # ------------------------------------------------ constants ------
        constf_t = const.tile([P, P + 1], f32)
        nc.sync.dma_start(constf_t[:], constf[:])
        iota = constf_t[:, 0:P]
        wt_t = const.tile([P, D], f16)
        nc.sync.dma_start(wt_t[:], wt[:])
        b_t = const.tile([1, D], f16)
        nc.sync.dma_start(b_t[:], bias[:])
        ones16 = const.tile([P, 1], f16)
        nc.vector.memset(ones16[:], 1.0)
        idxa_t = const.tile([P, SLOTS_A // 16], i16)
        nc.sync.dma_start(idxa_t[:], idxA[:])
        sega_t = const.tile([P, TA_tot], f32)
        nc.sync.dma_start(sega_t[:], segA[:])
        cnta_t = const.tile([P, TA_tot], i32)
        nc.sync.dma_start(cnta_t[:], cntA[:])
        cnte_t = const.tile([P, EBLK], i32)
        nc.sync.dma_start(cnte_t[:], cntE[:])
        idxb_t = const.tile([P, SLOTS_B // 16], i16)
        nc.sync.dma_start(idxb_t[:], idxB[:])
        segb_t = const.tile([P, TB_tot], f32)
        nc.sync.dma_start(segb_t[:], segB[:])
        cntb_t = const.tile([P, NBLK], i32)
        nc.sync.dma_start(cntb_t[:], cntB[:])

        # s per incidence = sqrt(1/cnt); inv_de = 1/cntE; s_n = sqrt(1/d_n)
        sa_t = const.tile([P, TA_tot], f32)
        nc.vector.tensor_copy(sa_t[:], cnta_t[:])
        nc.vector.reciprocal(sa_t[:], sa_t[:])
        nc.scalar.sqrt(sa_t[:], sa_t[:])
        inv_de = const.tile([P, EBLK], f32)
        nc.vector.tensor_copy(inv_de[:], cnte_t[:])
        nc.vector.reciprocal(inv_de[:], inv_de[:])
        sb_t = const.tile([P, NBLK], f32)
        nc.vector.tensor_copy(sb_t[:], cntb_t[:])
        nc.vector.reciprocal(sb_t[:], sb_t[:])
        nc.scalar.sqrt(sb_t[:], sb_t[:])

        # ------------------------------------------------ phase A --------
        co = [int(v) // P for v in cap_off]
        for blk in range(EBLK):
            g = gpool.tile([P, T_A, D], f16, tag="g")
            base16 = blk * (T_A * P) // 16
            for ch in range(n_chunks):
                cw = int(caps[ch])
                lo = CHUNK * ch
                hi = min(N, CHUNK * (ch + 1))
                nc.gpsimd.dma_gather(
                    out_ap=g[:, co[ch]:co[ch + 1], :],
                    in_ap=x[lo:hi, :],
                    idxs_ap=idxa_t[:, base16 + int(cap_off[ch]) // 16:
                                   base16 + int(cap_off[ch + 1]) // 16],
                    num_idxs=cw,
                    num_idxs_reg=cw,
                    elem_size=D,
                    single_packet=(cw <= 1024),
                )
            psum_p = pp.tile([P, P], f32, tag="pp")
            psum_q = pq.tile([1, P], f32, tag="pq")
            for t in range(T_A):
                tt = blk * T_A + t
                oh = ohpool.tile([P, P], f16, tag="oh")
                nc.vector.tensor_scalar(
                    out=oh[:], in0=iota,
                    scalar1=sega_t[:, tt:tt + 1], scalar2=sa_t[:, tt:tt + 1],
                    op0=mybir.AluOpType.is_equal, op1=mybir.AluOpType.mult)
                nc.tensor.matmul(psum_p[:], lhsT=g[:, t, :], rhs=oh[:],
                                 start=(t == 0), stop=(t == T_A - 1))
                nc.tensor.matmul(psum_q[:], lhsT=ones16[:], rhs=oh[:],
                                 start=(t == 0), stop=(t == T_A - 1))
            efp = epool.tile([P, P], f16, tag="efp")
            nc.scalar.activation(efp[:], psum_p[:],
                                 mybir.ActivationFunctionType.Copy)
            qrow = epool.tile([1, P], f16, tag="qrow")
            nc.vector.tensor_copy(qrow[:], psum_q[:])
            psum_e = pe.tile([P, P], f32, tag="pe")
            nc.tensor.matmul(psum_e[:], lhsT=efp[:], rhs=wt_t[:],
                             start=True, stop=False)
            nc.tensor.matmul(psum_e[:], lhsT=qrow[:], rhs=b_t[:],
                             start=False, stop=True)
            out_t = opool.tile([P, P], f16, tag="out")
            nc.scalar.activation(out_t[:], psum_e[:],
                                 mybir.ActivationFunctionType.Copy,
                                 scale=inv_de[:, blk:blk + 1])
            nc.sync.dma_start(ef_loc[blk * P:(blk + 1) * P, :], out_t[:])

        # ------------------------------------------------ exchange -------
        nc.gpsimd.collective_compute(
            "AllGather", mybir.AluOpType.bypass,
            replica_groups=[list(range(NCORE))],
            ins=[ef_loc.opt()], outs=[ef_full.opt()])

        # ------------------------------------------------ phase B --------
        nblk_grp = -(-NBLK // GB_B)
        for grp in range(nblk_grp):
            b0 = grp * GB_B
            nb = min(GB_B, NBLK - b0)
            g = gpoolB.tile([P, GB_B * TB, D], f16, tag="gB")
            nc.gpsimd.dma_gather(
                out_ap=g[:, 0:nb * TB, :],
                in_ap=ef_full[:],
                idxs_ap=idxb_t[:, b0 * TB * P // 16:(b0 + nb) * TB * P // 16],
                num_idxs=nb * TB * P,
                num_idxs_reg=nb * TB * P,
                elem_size=D,
                single_packet=(nb * TB * P <= 1024),
            )
            for bi in range(nb):
                blk = b0 + bi
                psum_y = py.tile([P, P], f32, tag="py")
                for t in range(TB):
                    tt = blk * TB + t
                    oh = ohpoolB.tile([P, P], f16, tag="ohB")
                    nc.vector.tensor_scalar(
                        out=oh[:], in0=iota,
                        scalar1=segb_t[:, tt:tt + 1], scalar2=None,
                        op0=mybir.AluOpType.is_equal)
                    nc.tensor.matmul(psum_y[:], lhsT=oh[:],
                                     rhs=g[:, bi * TB + t, :],
                                     start=(t == 0), stop=(t == TB - 1))
                out_t = opoolB.tile([P, P], f32, tag="outB")
                nc.scalar.activation(out_t[:], psum_y[:],
                                     mybir.ActivationFunctionType.Relu,
                                     scale=sb_t[:, blk:blk + 1])
                nc.sync.dma_start(y[blk * P:(blk + 1) * P, :], out_t[:])
    nc.compile()
    return nc


def _kernel_fused(x, W, b, N, pa, pb, NSLOT):
    EBLK, T_A = pa["EBLK"], pa["T_A"]
    NBLK, TB = pb["NBLK"], pb["TB"]

    key = ("F", N, EBLK, T_A, tuple(pa["caps"].tolist()), NSLOT, NBLK, TB)
    if key not in _NC_CACHE:
        _NC_CACHE[key] = _build_neff_fused(N, EBLK, T_A, pa["caps"],
                                           pa["cap_off"], pa["n_chunks"],
                                           NSLOT, NBLK, TB)
    nc = _NC_CACHE[key]

    iota = np.tile(np.arange(P, dtype=np.float32), (P, 1))
    constf = np.concatenate([iota, np.ones((P, 1), np.float32)], axis=1)
    x16 = x.astype(np.float16)
    wt16 = np.ascontiguousarray(W.T).astype(np.float16)
    b16 = b.astype(np.float16)

    in_maps = []
    for c in range(NCORE):
        in_maps.append({
            "x": x16, "wt": wt16, "constf": constf, "bias": b16,
            "idxA": pa["idxA"][c], "segA": pa["segA"][c],
            "cntA": pa["cntA"][c].astype(np.int32),
            "cntE": pa["cntE"][c].astype(np.int32),
            "idxB": pb["idxB"][c], "segB": pb["segB"][c],
            "cntB": pb["cntB"][c].astype(np.int32),
        })
    if TIME_RUNS:
        res = _with_retry(lambda: _timed_spmd(nc, in_maps, key))
    else:
        res = _with_retry(lambda: run_bass_kernel_spmd(
            nc, in_maps, core_ids=list(range(NCORE))))

    y_dev = np.stack([res.results[c]["y"] for c in range(NCORE)])
    out = y_dev[pb["core_of_node"],
                pb["blk_of_node"] * P + pb["slot_of_node"], :]
    return np.ascontiguousarray(out, dtype=np.float32)


# -------------------------------------------------------------------- main --
def kernel(x, W, b, node_idx, edge_idx, num_nodes=None, num_edges=None,
           **_ignored):
    x = np.asarray(x, np.float32)
    W = np.asarray(W, np.float32)
    b = np.asarray(b, np.float32).reshape(1, -1)
    node_idx = np.asarray(node_idx).astype(np.int64).ravel()
    edge_idx = np.asarray(edge_idx).astype(np.int64).ravel()
    N = int(num_nodes) if num_nodes is not None else x.shape[0]
    E = int(num_edges) if num_edges is not None else int(edge_idx.max()) + 1

    d_n = np.bincount(node_idx, minlength=N)
    d_e = np.bincount(edge_idx, minlength=E)

    pa = _pack_edges(node_idx, edge_idx, N, E, d_n, d_e)
    pb = _pack_nodes(node_idx, edge_idx, N, d_n,
                     pa["eslot"] )

    EBLK, T_A = pa["EBLK"], pa["T_A"]
    NBLK, TB = pb["NBLK"], pb["TB"]
    NSLOT = NCORE * EBLK * P

    if os.environ.get("HGNN_TWO_PHASE", "0") != "1":
        return _kernel_fused(x, W, b, N, pa, pb, NSLOT)

    keyA = ("A", N, EBLK, T_A, tuple(pa["caps"].tolist()))
    if keyA not in _NC_CACHE:
        _NC_CACHE[keyA] = _build_neff_a(N, EBLK, T_A, pa["caps"],
                                        pa["cap_off"], pa["n_chunks"])
    ncA = _NC_CACHE[keyA]
    keyB = ("B", NSLOT, NBLK, TB)
    if keyB not in _NC_CACHE:
        _NC_CACHE[keyB] = _build_neff_b(NSLOT, NBLK, TB)
    ncB = _NC_CACHE[keyB]

    iota = np.tile(np.arange(P, dtype=np.float32), (P, 1))
    constfA = np.concatenate([iota, np.ones((P, 1), np.float32)], axis=1)
    x16 = x.astype(np.float16)
    wt16 = np.ascontiguousarray(W.T).astype(np.float16)
    b16 = b.astype(np.float16)

    in_maps_a = []
    for c in range(NCORE):
        in_maps_a.append({
            "x": x16, "wt": wt16, "constf": constfA, "bias": b16,
            "idxA": pa["idxA"][c], "segA": pa["segA"][c],
            "cntA": pa["cntA"][c].astype(np.int32),
            "cntE": pa["cntE"][c].astype(np.int32),
        })
    if TIME_RUNS:
        resA = _timed_spmd(ncA, in_maps_a, keyA)
    else:
        resA = run_bass_kernel_spmd(ncA, in_maps_a,
                                    core_ids=list(range(NCORE)))

    ef_full = np.concatenate([resA.results[c]["ef"] for c in range(NCORE)],
                             axis=0).astype(np.float16)
    assert ef_full.shape == (NSLOT, D)

    in_maps_b = []
    for c in range(NCORE):
        in_maps_b.append({
            "ef": ef_full, "constf": iota,
            "idxB": pb["idxB"][c], "segB": pb["segB"][c],
            "cntB": pb["cntB"][c].astype(np.int32),
        })
    if TIME_RUNS:
        resB = _timed_spmd(ncB, in_maps_b, keyB)
    else:
        resB = run_bass_kernel_spmd(ncB, in_maps_b,
                                    core_ids=list(range(NCORE)))

    y_dev = np.stack([resB.results[c]["y"] for c in range(NCORE)])
    out = y_dev[pb["core_of_node"],
                pb["blk_of_node"] * P + pb["slot_of_node"], :]
    return np.ascontiguousarray(out, dtype=np.float32)

